# revision 1
# baseline (speedup 1.0000x reference)
"""Trainium2 Bass kernel for DETR PartMap mask generation.

Reference computes, for each (set, batch, query) with set in {object, subject}:
    x1 = floor((cx - w/2) * img_w / 32); x2 = min(floor((cx + w/2) * img_w / 32), 39)
    y1 = floor((cy - h/2) * img_h / 32); y2 = min(floor((cy + h/2) * img_h / 32), 39)
    mask[h, w] = (h < y1) | (h > y2) | (w < x1) | (w > x2)   (as float32)
Output: [2, 64, 300, 40, 40] float32  (~245 MB -> purely output-DMA bound).

Key identities (exact for integer grid j in [0, 39], any float v, since all
involved quantities are exactly representable in f32):
    j < floor(v)            <=>  j <= v - 1
    j > min(floor(v), 39)   <=>  j > v          (because j <= 39 always)
so no floor/int-cast is needed on device; compares on raw f32 products are
bit-identical to the reference.

Per-(b,q) masks are separable: mask[h,w] = row_out[h] OR col_out[w].
Per 120-query tile we compute two [120,40] vectors (4 small vector ops) and
combine them with ONE [120,1600] broadcast tensor_tensor(max) op, then DMA
the 768 KB tile straight out. Data-parallel over batch across 8 cores.
"""

import numpy as np

B, Q, HM, WM = 64, 300, 40, 40
NCORES = 8
BC = B // NCORES            # batches per core
NSET = 2                    # object / subject
ROWS = NSET * BC * Q        # 4800 rows per core
P = 120                     # partitions per tile (4800 = 40 * 120)
NT = ROWS // P              # 40 tiles
FREE = HM * WM              # 1600

_CACHE = {}


def _build_nc():
    from contextlib import ExitStack

    import concourse.bass as bass  # noqa: F401
    import concourse.tile as tile
    from concourse import bacc, mybir

    f32 = mybir.dt.float32
    Alu = mybir.AluOpType

    nc = bacc.Bacc("TRN2", target_bir_lowering=False, debug=False)

    # packed input: 7 planes of [P, NT]: cx, cy, w, h, sw, sh, grid
    pk_d = nc.dram_tensor("pk", [P, 7 * NT], f32, kind="ExternalInput")
    out_d = nc.dram_tensor("out", [ROWS, FREE], f32, kind="ExternalOutput")

    with tile.TileContext(nc) as tc:
        with ExitStack() as ctx:
            cpool = ctx.enter_context(tc.tile_pool(name="const", bufs=1))
            spool = ctx.enter_context(tc.tile_pool(name="small", bufs=4))
            opool = ctx.enter_context(tc.tile_pool(name="obuf", bufs=8))

            pk = cpool.tile([P, 7 * NT], f32)
            nc.sync.dma_start(pk[:], pk_d.ap())
            CX = pk[:, 0 * NT : 1 * NT]
            CY = pk[:, 1 * NT : 2 * NT]
            Wp = pk[:, 2 * NT : 3 * NT]
            Hp = pk[:, 3 * NT : 4 * NT]
            SW = pk[:, 4 * NT : 5 * NT]
            SH = pk[:, 5 * NT : 6 * NT]
            G = pk[:, 6 * NT : 7 * NT]  # grid: G[p, j] = j

            # batched box-param math, all [P, NT] f32, rounding-identical
            # to the reference: t1 = c -/+ 0.5*s ; t2 = t1 * size ;
            # bound = t2 * (1/32) (exact);  lower bound additionally -1.
            pr = cpool.tile([P, 4 * NT], f32)
            X1M = pr[:, 0 * NT : 1 * NT]
            X2F = pr[:, 1 * NT : 2 * NT]
            Y1M = pr[:, 2 * NT : 3 * NT]
            Y2F = pr[:, 3 * NT : 4 * NT]

            tmp = cpool.tile([P, 6 * NT], f32)
            HW2 = tmp[:, 0 * NT : 1 * NT]
            HH2 = tmp[:, 1 * NT : 2 * NT]
            T1 = tmp[:, 2 * NT : 3 * NT]
            T2 = tmp[:, 3 * NT : 4 * NT]
            T3 = tmp[:, 4 * NT : 5 * NT]
            T4 = tmp[:, 5 * NT : 6 * NT]

            v = nc.vector
            v.tensor_scalar(HW2, Wp, 0.5, None, Alu.mult)
            v.tensor_scalar(HH2, Hp, 0.5, None, Alu.mult)
            v.tensor_tensor(T1, CX, HW2, Alu.subtract)   # x1c
            v.tensor_tensor(T2, CX, HW2, Alu.add)        # x2c
            v.tensor_tensor(T3, CY, HH2, Alu.subtract)   # y1c
            v.tensor_tensor(T4, CY, HH2, Alu.add)        # y2c
            v.tensor_tensor(T1, T1, SW, Alu.mult)        # x1c*sw
            v.tensor_tensor(T2, T2, SW, Alu.mult)
            v.tensor_tensor(T3, T3, SH, Alu.mult)
            v.tensor_tensor(T4, T4, SH, Alu.mult)
            v.tensor_scalar(X1M, T1, 1.0 / 32.0, 1.0, Alu.mult, Alu.subtract)
            v.tensor_scalar(X2F, T2, 1.0 / 32.0, None, Alu.mult)
            v.tensor_scalar(Y1M, T3, 1.0 / 32.0, 1.0, Alu.mult, Alu.subtract)
            v.tensor_scalar(Y2F, T4, 1.0 / 32.0, None, Alu.mult)

            for t in range(NT):
                a = spool.tile([P, HM], f32, tag="a")
                v.tensor_scalar(a[:], G, Y1M[:, t : t + 1], None, Alu.is_le)
                rowm = spool.tile([P, HM], f32, tag="rowm")
                v.scalar_tensor_tensor(
                    rowm[:], G, Y2F[:, t : t + 1], a[:], Alu.is_gt, Alu.logical_or
                )
                c = spool.tile([P, WM], f32, tag="c")
                v.tensor_scalar(c[:], G, X1M[:, t : t + 1], None, Alu.is_le)
                colm = spool.tile([P, WM], f32, tag="colm")
                v.scalar_tensor_tensor(
                    colm[:], G, X2F[:, t : t + 1], c[:], Alu.is_gt, Alu.logical_or
                )

                omt = opool.tile([P, FREE], f32, tag="o")
                rb = rowm[:].unsqueeze(2).broadcast_to([P, HM, WM])
                cb = colm[:].unsqueeze(1).broadcast_to([P, HM, WM])
                ov = omt[:].rearrange("p (h w) -> p h w", w=WM)
                nc.any.tensor_tensor(ov, rb, cb, Alu.max)

                nc.sync.dma_start(out_d.ap()[t * P : (t + 1) * P, :], omt[:])

    nc.compile()
    return nc


def _get_nc():
    if "nc" not in _CACHE:
        _CACHE["nc"] = _build_nc()
    return _CACHE["nc"]


def _plane(v):
    """[ROWS] host vector -> [P, NT] sbuf layout with row r = t*P + p."""
    return np.ascontiguousarray(v.reshape(NT, P).T)


def _pack_core(coords_c, sw_c, sh_c, grid):
    """coords_c: [NSET, BC, Q, 4] f32; sw_c/sh_c: [BC] f32 -> pk [P, 7*NT]."""
    rows = coords_c.reshape(ROWS, 4)
    sw_rows = np.tile(np.repeat(sw_c, Q), NSET)
    sh_rows = np.tile(np.repeat(sh_c, Q), NSET)
    planes = [
        _plane(rows[:, 0]),
        _plane(rows[:, 1]),
        _plane(rows[:, 2]),
        _plane(rows[:, 3]),
        _plane(sw_rows),
        _plane(sh_rows),
        grid,
    ]
    return np.ascontiguousarray(np.concatenate(planes, axis=1), dtype=np.float32)


def make_in_maps(obj_coord, sub_coord, img_size):
    coords = np.stack(
        [np.asarray(obj_coord, np.float32), np.asarray(sub_coord, np.float32)], axis=0
    )  # [2, B, Q, 4]
    szf = np.asarray(img_size).astype(np.float32)  # [B, 2] = (h, w)
    sh, sw = szf[:, 0], szf[:, 1]
    grid = np.tile(np.arange(NT, dtype=np.float32), (P, 1))
    in_maps = []
    for core in range(NCORES):
        b0, b1 = core * BC, (core + 1) * BC
        pk = _pack_core(coords[:, b0:b1], sw[b0:b1], sh[b0:b1], grid)
        in_maps.append({"pk": pk})
    return in_maps


def kernel(obj_coord, sub_coord, img_size, mask):
    from concourse.bass_utils import run_bass_kernel_spmd

    nc = _get_nc()
    in_maps = make_in_maps(obj_coord, sub_coord, img_size)
    res = run_bass_kernel_spmd(nc, in_maps, core_ids=list(range(NCORES)))
    parts = [
        res.results[core]["out"].reshape(NSET, BC, Q, HM, WM) for core in range(NCORES)
    ]
    return np.ascontiguousarray(np.concatenate(parts, axis=1), dtype=np.float32)


# revision 6
# speedup vs baseline: 108431.9684x; 108431.9684x over previous
"""Trainium2 Bass kernel for DETR PartMap mask generation.

Reference computes, for each (set, batch, query) with set in {object, subject}:
    x1 = floor((cx - w/2) * img_w / 32); x2 = min(floor((cx + w/2) * img_w / 32), 39)
    y1 = floor((cy - h/2) * img_h / 32); y2 = min(floor((cy + h/2) * img_h / 32), 39)
    mask[h, w] = (h < y1) | (h > y2) | (w < x1) | (w > x2)   (as float32)
Output: [2, 64, 300, 40, 40] float32  (~245 MB -> purely output-DMA bound).

Key identities (exact for integer grid j in [0, 39], any float v, since all
involved quantities are exactly representable in f32):
    j < floor(v)            <=>  j <= v - 1
    j > min(floor(v), 39)   <=>  j > v          (because j <= 39 always)
so no floor/int-cast is needed on device; compares on raw f32 products are
bit-identical to the reference.

Per-(b,q) masks are separable: mask[h,w] = row_out[h] OR col_out[w].
Per 120-query tile we compute two [120,40] vectors (4 small vector ops) and
combine them with ONE [120,1600] broadcast tensor_tensor(max) op, then DMA
the 768 KB tile straight out. Data-parallel over batch across 8 cores.
"""

import numpy as np

B, Q, HM, WM = 64, 300, 40, 40
NCORES = 8
BC = B // NCORES            # batches per core
NSET = 2                    # object / subject
ROWS = NSET * BC * Q        # 4800 rows per core
P = 120                     # partitions per tile (4800 = 40 * 120)
NT = ROWS // P              # 40 tiles
FREE = HM * WM              # 1600

_CACHE = {}


def _build_nc():
    from contextlib import ExitStack

    import concourse.bass as bass  # noqa: F401
    import concourse.tile as tile
    from concourse import bacc, mybir

    f32 = mybir.dt.float32
    Alu = mybir.AluOpType

    nc = bacc.Bacc("TRN2", target_bir_lowering=False, debug=False)

    # packed input: 7 planes of [P, NT]: cx, cy, w, h, sw, sh, grid
    pk_d = nc.dram_tensor("pk", [P, 7 * NT], f32, kind="ExternalInput")
    out_d = nc.dram_tensor("out", [ROWS, FREE], f32, kind="ExternalOutput")

    with tile.TileContext(nc) as tc:
        with ExitStack() as ctx:
            cpool = ctx.enter_context(tc.tile_pool(name="const", bufs=1))
            spool = ctx.enter_context(tc.tile_pool(name="small", bufs=4))
            opool = ctx.enter_context(tc.tile_pool(name="obuf", bufs=16))

            pk = cpool.tile([P, 7 * NT], f32)
            nc.sync.dma_start(pk[:], pk_d.ap())
            CX = pk[:, 0 * NT : 1 * NT]
            CY = pk[:, 1 * NT : 2 * NT]
            Wp = pk[:, 2 * NT : 3 * NT]
            Hp = pk[:, 3 * NT : 4 * NT]
            SW = pk[:, 4 * NT : 5 * NT]
            SH = pk[:, 5 * NT : 6 * NT]
            G = pk[:, 6 * NT : 7 * NT]  # grid: G[p, j] = j

            # batched box-param math, all [P, NT] f32, rounding-identical
            # to the reference: t1 = c -/+ 0.5*s ; t2 = t1 * size ;
            # bound = t2 * (1/32) (exact);  lower bound additionally -1.
            pr = cpool.tile([P, 4 * NT], f32)
            X1M = pr[:, 0 * NT : 1 * NT]
            X2F = pr[:, 1 * NT : 2 * NT]
            Y1M = pr[:, 2 * NT : 3 * NT]
            Y2F = pr[:, 3 * NT : 4 * NT]

            tmp = cpool.tile([P, 6 * NT], f32)
            HW2 = tmp[:, 0 * NT : 1 * NT]
            HH2 = tmp[:, 1 * NT : 2 * NT]
            T1 = tmp[:, 2 * NT : 3 * NT]
            T2 = tmp[:, 3 * NT : 4 * NT]
            T3 = tmp[:, 4 * NT : 5 * NT]
            T4 = tmp[:, 5 * NT : 6 * NT]

            v = nc.vector
            v.tensor_scalar(HW2, Wp, 0.5, None, Alu.mult)
            v.tensor_scalar(HH2, Hp, 0.5, None, Alu.mult)
            v.tensor_tensor(T1, CX, HW2, Alu.subtract)   # x1c
            v.tensor_tensor(T2, CX, HW2, Alu.add)        # x2c
            v.tensor_tensor(T3, CY, HH2, Alu.subtract)   # y1c
            v.tensor_tensor(T4, CY, HH2, Alu.add)        # y2c
            v.tensor_tensor(T1, T1, SW, Alu.mult)        # x1c*sw
            v.tensor_tensor(T2, T2, SW, Alu.mult)
            v.tensor_tensor(T3, T3, SH, Alu.mult)
            v.tensor_tensor(T4, T4, SH, Alu.mult)
            v.tensor_scalar(X1M, T1, 1.0 / 32.0, 1.0, Alu.mult, Alu.subtract)
            v.tensor_scalar(X2F, T2, 1.0 / 32.0, None, Alu.mult)
            v.tensor_scalar(Y1M, T3, 1.0 / 32.0, 1.0, Alu.mult, Alu.subtract)
            v.tensor_scalar(Y2F, T4, 1.0 / 32.0, None, Alu.mult)

            for t in range(NT):
                a = spool.tile([P, HM], f32, tag="a")
                v.tensor_scalar(a[:], G, Y1M[:, t : t + 1], None, Alu.is_le)
                rowm = spool.tile([P, HM], f32, tag="rowm")
                v.scalar_tensor_tensor(
                    rowm[:], G, Y2F[:, t : t + 1], a[:], Alu.is_gt, Alu.logical_or
                )
                c = spool.tile([P, WM], f32, tag="c")
                v.tensor_scalar(c[:], G, X1M[:, t : t + 1], None, Alu.is_le)
                colm = spool.tile([P, WM], f32, tag="colm")
                v.scalar_tensor_tensor(
                    colm[:], G, X2F[:, t : t + 1], c[:], Alu.is_gt, Alu.logical_or
                )

                omt = opool.tile([P, FREE], f32, tag="o")
                rb = rowm[:].unsqueeze(2).broadcast_to([P, HM, WM])
                cb = colm[:].unsqueeze(1).broadcast_to([P, HM, WM])
                ov = omt[:].rearrange("p (h w) -> p h w", w=WM)
                # big combine (TensorTensor is only ISA-legal on DVE)
                nc.vector.tensor_tensor(ov, rb, cb, Alu.max)

                # alternate output DMAs across the two HWDGE rings (SP/ACT)
                dma_eng = nc.scalar if t % 2 else nc.sync
                dma_eng.dma_start(out_d.ap()[t * P : (t + 1) * P, :], omt[:])

    nc.compile()
    return nc


def _get_nc():
    if "nc" not in _CACHE:
        _CACHE["nc"] = _build_nc()
    return _CACHE["nc"]


def _plane(v):
    """[ROWS] host vector -> [P, NT] sbuf layout with row r = t*P + p."""
    return np.ascontiguousarray(v.reshape(NT, P).T)


def _pack_core(coords_c, sw_c, sh_c, grid):
    """coords_c: [NSET, BC, Q, 4] f32; sw_c/sh_c: [BC] f32 -> pk [P, 7*NT]."""
    rows = coords_c.reshape(ROWS, 4)
    sw_rows = np.tile(np.repeat(sw_c, Q), NSET)
    sh_rows = np.tile(np.repeat(sh_c, Q), NSET)
    planes = [
        _plane(rows[:, 0]),
        _plane(rows[:, 1]),
        _plane(rows[:, 2]),
        _plane(rows[:, 3]),
        _plane(sw_rows),
        _plane(sh_rows),
        grid,
    ]
    return np.ascontiguousarray(np.concatenate(planes, axis=1), dtype=np.float32)


def make_in_maps(obj_coord, sub_coord, img_size):
    coords = np.stack(
        [np.asarray(obj_coord, np.float32), np.asarray(sub_coord, np.float32)], axis=0
    )  # [2, B, Q, 4]
    szf = np.asarray(img_size).astype(np.float32)  # [B, 2] = (h, w)
    sh, sw = szf[:, 0], szf[:, 1]
    grid = np.tile(np.arange(NT, dtype=np.float32), (P, 1))
    in_maps = []
    for core in range(NCORES):
        b0, b1 = core * BC, (core + 1) * BC
        pk = _pack_core(coords[:, b0:b1], sw[b0:b1], sh[b0:b1], grid)
        in_maps.append({"pk": pk})
    return in_maps


def _run_cores(nc, in_maps):
    """First call goes through bass_utils.run_bass_kernel_spmd (compiles the
    NEFF); on native hardware later calls re-execute the cached NEFF via
    bass_utils.run_neff to avoid recompiling per call."""
    from concourse import bass_utils
    from concourse._compat import axon_active

    core_ids = list(range(NCORES))
    if axon_active():
        res = bass_utils.run_bass_kernel_spmd(nc, in_maps, core_ids=core_ids)
        return [r["out"] for r in res.results]

    if "neff_tmpdir" not in _CACHE:
        import tempfile

        tmpdir = tempfile.mkdtemp(prefix="partmap_neff_")
        res = bass_utils.run_bass_kernel_spmd(
            nc, in_maps, core_ids=core_ids, tmpdir=tmpdir
        )
        _CACHE["neff_tmpdir"] = tmpdir
        return [r["out"] for r in res.results]

    import glob as _glob

    neff = _glob.glob(_CACHE["neff_tmpdir"] + "/sg00/*.neff")
    if not neff:
        res = bass_utils.run_bass_kernel_spmd(nc, in_maps, core_ids=core_ids)
        return [r["out"] for r in res.results]
    out_maps = [
        {"out": np.zeros((ROWS, FREE), np.float32)} for _ in core_ids
    ]
    results = bass_utils.run_neff(neff[0], in_maps, out_maps, core_ids)
    return [r["out"] for r in results]


def kernel(obj_coord, sub_coord, img_size, mask):
    nc = _get_nc()
    in_maps = make_in_maps(obj_coord, sub_coord, img_size)
    outs = _run_cores(nc, in_maps)
    parts = [o.reshape(NSET, BC, Q, HM, WM) for o in outs]
    return np.ascontiguousarray(np.concatenate(parts, axis=1), dtype=np.float32)
